# revision 35
# baseline (speedup 1.0000x reference)
"""AttnConv2d Trainium2 kernel (8-core SPMD, data-parallel over batch).

Problem (per sample b):
  CK  = conv3x3_same(x1, key_w);  CQ1 = conv3x3_same(x1, q1_w);  CQ2 = conv3x3_same(x2, q2_w)
  For each kernel position p (3x3 grid phase) the conv outputs are unfolded with
  stride 3 -> per-p pixel sets of L=1024 pixels.
  A1[o,c,p] = sum_l CQ1[o,pix] * CK[c,pix];  A2 likewise with CQ2.
  attn = (A1+pos1)*(A2+pos2);  global mean/std normalize (over the whole batch);
  out  = conv3x3_same(x1, attn reshaped to [Cout,Cin,3,3] per sample).

Device strategy (one sample per NeuronCore):
  - All matmuls in bfloat16 (inputs bf16, PSUM accumulation fp32). End-to-end
    numpy simulation of bf16 rounding at every matmul interface gives
    max-normalized error ~6e-3, comfortably under the 2e-2 gate.
  - Convs computed in transposed layout [pixel, channel] (lhsT = shifted image
    band, rhs = weights), tiled so each 128-pixel tile belongs to one kernel
    position p; A accumulated as A^T[c, p*256+o] via PSUM + DVE adds.
  - The A^T (a12) matmuls are software-pipelined one p-step behind the conv
    matmuls so the PE never waits on the PSUM->SBUF copies feeding them.
  - During the last m iteration the per-p attn slices are multiplied as soon
    as their accumulation completes, so phase C can start immediately.
  - Normalization is deferred via linearity: the device returns the
    UN-normalized U = conv(x1, attn_raw) plus per-core attn sum/sumsq partials.
    Host computes global a,b and finishes out = a*U + b*boxsum(x1) (boxsum is
    conv(x1, ones) = the contribution of the constant shift b to the conv).
"""

import os
import sys
from contextlib import ExitStack

import numpy as np
import ml_dtypes

for _p in ("/opt/trn_rl_repo",):
    if _p not in sys.path and os.path.isdir(_p):
        sys.path.append(_p)

import concourse.bacc as bacc
import concourse.tile as tile
import concourse.mybir as mybir
from concourse.bass_utils import run_bass_kernel_spmd

F32 = mybir.dt.float32
BF16 = mybir.dt.bfloat16
BF16_NP = ml_dtypes.bfloat16

B, CIN, COUT, H, W = 8, 256, 256, 96, 96
K = 3
K2 = K * K
HW = H * W          # 9216
NORM_SCALE = 1.0
N_CORES = 8

_CACHE = {}


def build_nc():
    nc = bacc.Bacc("TRN2", target_bir_lowering=False, debug=False,
                   enable_asserts=True, num_devices=N_CORES)
    x1 = nc.dram_tensor("x1", [CIN, H, W], BF16, kind="ExternalInput")
    x2 = nc.dram_tensor("x2", [CIN, H, W], BF16, kind="ExternalInput")
    wkq1 = nc.dram_tensor("wkq1", [CIN, K2 * 2 * COUT], BF16, kind="ExternalInput")
    wq2 = nc.dram_tensor("wq2", [CIN, K2 * COUT], BF16, kind="ExternalInput")
    pos1t = nc.dram_tensor("pos1t", [CIN, K2 * COUT], F32, kind="ExternalInput")
    pos2t = nc.dram_tensor("pos2t", [CIN, K2 * COUT], F32, kind="ExternalInput")
    u_out = nc.dram_tensor("u", [COUT, HW], BF16, kind="ExternalOutput")
    stats_out = nc.dram_tensor("stats", [128, 40], F32, kind="ExternalOutput")

    with tile.TileContext(nc) as tc:
        _emit(nc, tc, x1, x2, wkq1, wq2, pos1t, pos2t, u_out, stats_out)
    nc.compile()
    return nc


def _strided_view(ap, free_dims, offset_elems):
    """Custom strided free-dim view of an SBUF AP (keeps partition dim)."""
    v = ap.copy()
    a = v.ap
    while len(a) > 1:
        a.pop()
    for d in free_dims:
        a.append(list(d))
    v.ap = a
    v.offset = ap.offset + offset_elems
    return v


# lhsT block-start row in the per-phase band for tap offset v = dy+ty-1:
#   dy'=v%3 phase blocks: dy'=0 at rows 0..4 (ly' 4m..4m+4), dy'=1 at rows
#   5..8 (ly' 4m..4m+3), dy'=2 at rows 9..13 (ly' 4m-1..4m+3)
_SROW = {-1: 9, 0: 0, 1: 5, 2: 10, 3: 1}
# per-phase (dyp -> (block base, n rows, staging row of local r=0, ...)):
#   staging row of phase-local row r:  dy'=0: 3r+1, dy'=1: 3r+2, dy'=2: 3r
_PHASE_ROWS = {0: (0, 5, 1), 1: (5, 4, 2), 2: (9, 5, 0)}


def _phase_b(nc, tc, ctx, x1, x2, wkq1, wq2, a1t, a2t, pos1t, pos2t,
             attn_p, cb, CBR):
    """Convs in [pixel, channel] layout + A^T accumulation.

    Rowwise compact staging band (96-wide, single contiguous DMA run per
    partition) is DMA'd from DRAM, then engines copy-split it into per-phase
    grids (the x=-1/x=96 pad columns are never-overwritten zeros in the band):
      band[c, j, dxx, drow, lx] = x[j*128+c, y(drow), 3*lx + (dxx-1)]
    so a tap (p=(dy,dx), t=(ty,tx)) reads the CONTIGUOUS 128-px block
    band[:, j, dx+tx, SROW[dy+ty-1] : +4, :]  (4 subgrid rows x 32).

    The a12 (A^T) matmuls for p-step k are emitted during step k+1 so the
    PE never stalls on the DVE copies (ckt/q12t) that feed them.
    """
    wpool = ctx.enter_context(tc.tile_pool(name="weights", bufs=1))
    stpool = ctx.enter_context(tc.tile_pool(name="staging", bufs=1))
    bpool = ctx.enter_context(tc.tile_pool(name="bands", bufs=1))
    cpool = ctx.enter_context(tc.tile_pool(name="convsb", bufs=2))
    # convps pinned to the LEFT psum banks and aps to the RIGHT: phase C's
    # first cps buffer (side=left) then reuses convps's banks, whose last
    # readers (the PSUM->SBUF casts) finish ~2.5us before aps's readers
    # (the final adds), so phase C's first matmul starts that much earlier
    aps = ctx.enter_context(tc.tile_pool(name="aps", bufs=2, space="PSUM",
                                          side="right"))
    convps = ctx.enter_context(tc.tile_pool(name="convps", bufs=2, space="PSUM",
                                            side="left"))

    SR = 14
    # staging is COMPACT 96-wide (no left/right zero pad): the DMA from DRAM
    # is then one contiguous multi-KB run per partition instead of 192-byte
    # rows (which cost 2.5-6.5us of DMA-queue time each). The x=-1 / x=96
    # zero columns live only in the band tiles as never-overwritten zeros.
    st1 = stpool.tile([128, 2, SR, 96], BF16, tag="st1")
    st2 = stpool.tile([128, 2, SR, 96], BF16, tag="st2")
    for stg in (st1, st2):
        # row 0 only (x row y=-1 at m=0): every later m overwrites it with
        # real data, and row SR-1 is DMA-covered at m=0 and re-zeroed at m=7
        nc.vector.memset(stg[:, :, 0, :], 0.0)
    # wkq1: [cin, tau, 512] = [wk_tau | wq1_tau] packed on host.
    # Each dma_start costs its issuing engine ~0.7us, so chunking is coarse:
    # j=0 in 2 pieces per tensor on the ACT queue (first matmuls only wait on
    # the t0-2 piece); j=1 whole on the SP queue behind the m=0 staging.
    wkq1_t = wpool.tile([128, 2, K2 * 2 * COUT], BF16, tag="wkq1")
    wq2_t = wpool.tile([128, 2, K2 * COUT], BF16, tag="wq2")
    nc.scalar.dma_start(wkq1_t[:, 0, 0:3 * 512], wkq1[0:128, 0:3 * 512])
    nc.scalar.dma_start(wq2_t[:, 0, 0:3 * COUT], wq2[0:128, 0:3 * COUT])
    nc.scalar.dma_start(wkq1_t[:, 0, 3 * 512:K2 * 512], wkq1[0:128, 3 * 512:K2 * 512])
    nc.scalar.dma_start(wq2_t[:, 0, 3 * COUT:K2 * COUT], wq2[0:128, 3 * COUT:K2 * COUT])
    # HAM warm-up: ~20 dependency-free matmuls on zeroed scratch run in the
    # otherwise-idle window between the engine preamble and the first real
    # conv (weights still streaming from HBM), so the PE clock is already at
    # 8/8 when the real work starts instead of ramping through it
    warm = wpool.tile([128, 512], BF16, tag="warm")
    warm_ps = aps.tile([128, 2, 512], F32, tag="a12_ps")
    nc.vector.memset(warm[:], 0.0)
    for _ in range(32):
        nc.tensor.matmul(warm_ps[:, 0, :], warm[:, 0:128], warm[:],
                         start=True, stop=True)
    x1b = [bpool.tile([128, 2, 5, SR, 32], BF16, tag=f"x1b{i}", name=f"x1b{i}") for i in range(2)]
    x2b = [bpool.tile([128, 2, 5, SR, 32], BF16, tag=f"x2b{i}", name=f"x2b{i}") for i in range(2)]
    for band in (*x1b, *x2b):
        # x=-1 (dxx=0, lx=0) and x=96 (dxx=4, lx=31) zeros: the split copies
        # below never write these elements, so a single startup memset holds
        # for all 8 m iterations
        nc.vector.memset(band[:, :, 0, :, 0], 0.0)
        nc.vector.memset(band[:, :, 4, :, 31], 0.0)

    # deferred-a12 pipeline state: (m, p, ckt, q12t) of the previous step
    pend = [None]

    def flush_pend():
        if pend[0] is None:
            return
        pm, pp, ckt, q12t = pend[0]
        psl = slice(pp * COUT, (pp + 1) * COUT)
        a12_ps = aps.tile([128, 2, 512], F32, tag="a12_ps")
        for j in range(2):
            nc.tensor.matmul(a12_ps[:, j, :], ckt[:, j * 128:(j + 1) * 128], q12t[:],
                             start=True, stop=True)
        nc.vector.tensor_tensor(out=a1t[:, :, psl], in0=a1t[:, :, psl],
                                in1=a12_ps[:, :, 0:COUT], op=mybir.AluOpType.add)
        nc.vector.tensor_tensor(out=a2t[:, :, psl], in0=a2t[:, :, psl],
                                in1=a12_ps[:, :, COUT:2 * COUT], op=mybir.AluOpType.add)
        if pm == 7:
            # finalize this p slice of attn immediately so phase C's
            # stationary reads unblock long before phase B fully drains;
            # each p gets its OWN tile so the dependency stays precise
            # (9 slice-writes into one tile made Tile coarsen the dep and
            # phase C's first weight load waited on the LAST write)
            nc.vector.tensor_tensor(out=attn_p[pp][:, :, :], in0=a1t[:, :, psl],
                                    in1=a2t[:, :, psl], op=mybir.AluOpType.mult)
        pend[0] = None

    for m in range(8):
        bx1, bx2 = x1b[m % 2], x2b[m % 2]
        y0 = 12 * m - 1                       # img row of staging row 0
        r0, r1 = max(0, -y0), min(SR, 96 - y0)
        if m == 7:
            # m=6 filled staging row 13 with real data; y=96 must be zero
            nc.gpsimd.memset(st1[:, :, SR - 1, :], 0.0)
            nc.gpsimd.memset(st2[:, :, SR - 1, :], 0.0)
            # prefetch phase-C bands for groups 0 and 1 (pure x1 reload, no
            # dependency on phase B compute; SP queue is idle by now)
            for g, gy in ((0, 0), (1, 20)):
                cy0 = gy - 1
                cr0 = max(0, -cy0)
                cr1 = min(CBR, 96 - cy0, 22)
                for j in range(2):
                    nc.sync.dma_start(cb[g][:, j, cr0:cr1, 1:97],
                                      x1[j * 128:(j + 1) * 128, cy0 + cr0:cy0 + cr1, :])
        # j=0 staging lands first so the very first conv matmuls (which only
        # touch the j=0 cin half) can start before j=1 arrives; high_priority
        # at m=0 keeps the scheduler from hoisting the wait-free j1-weight
        # DMA ahead of these on the SP queue
        import contextlib
        with tc.high_priority() if m == 0 else contextlib.nullcontext():
            for j in range(2):
                for stg, xdram in ((st1, x1), (st2, x2)):
                    nc.sync.dma_start(stg[:, j, r0:r1, :],
                                      xdram[j * 128:(j + 1) * 128, y0 + r0:y0 + r1, :])
        if m == 0:
            # j=1 weights + pos accumulator init on the SP queue behind the
            # m=0 staging: they're needed mid-p0 / at the p0 adds, well after
            # the staging, and keep the ACT queue down to 4 descriptors
            # j=1 weights in two pieces each: the t0-2 chunks unblock the
            # first j=1 taps ~1.5us before the bulk lands
            nc.sync.dma_start(wkq1_t[:, 1, 0:3 * 512], wkq1[128:256, 0:3 * 512])
            nc.sync.dma_start(wq2_t[:, 1, 0:3 * COUT], wq2[128:256, 0:3 * COUT])
            nc.sync.dma_start(wkq1_t[:, 1, 3 * 512:K2 * 512],
                              wkq1[128:256, 3 * 512:K2 * 512])
            nc.sync.dma_start(wq2_t[:, 1, 3 * COUT:K2 * COUT],
                              wq2[128:256, 3 * COUT:K2 * COUT])
            for j in range(2):
                nc.sync.dma_start(a1t[:, j, :], pos1t[j * 128:(j + 1) * 128, :])
                nc.sync.dma_start(a2t[:, j, :], pos2t[j * 128:(j + 1) * 128, :])
        if m == 5:
            # phase-C band edge zeros, emitted here (DVE, ~100ns each) so they
            # neither sit on the startup critical path nor delay the m=7
            # prefetch DMAs that depend on them
            for bt in cb:
                nc.vector.memset(bt[:, :, 0, :], 0.0)
                nc.vector.memset(bt[:, :, :, 0], 0.0)
                nc.vector.memset(bt[:, :, :, 97], 0.0)
        # phase-split copies: staging -> band, spread across idle engines
        # band[c, j, dxx, base+r, lx] = staging[c, j, srow0+3r, 3*lx + dxx - 1]
        # (x-index 3*lx+dxx-1; the out-of-range x=-1 / x=96 elements are the
        # startup band zeros, so dxx=0 copies lx>=1 and dxx=4 copies lx<=30)
        def _cp(eng, dst, src):
            if eng is nc.scalar:
                nc.scalar.copy(dst, src)
            else:
                eng.tensor_copy(dst, src)

        def split_copy(engs3, stg, band, j, dyp):
            base, nr, srow0 = _PHASE_ROWS[dyp]
            sv = stg[:, j, :, :]                               # [128, SR, 96]
            _cp(engs3[1], band[:, j, 0, base:base + nr, 1:32],
                _strided_view(sv, [[3 * 96, nr], [3, 31]], srow0 * 96 + 2))
            _cp(engs3[0], band[:, j, 1:4, base:base + nr, :],
                _strided_view(sv, [[1, 3], [3 * 96, nr], [3, 32]], srow0 * 96))
            _cp(engs3[2], band[:, j, 4, base:base + nr, 0:31],
                _strided_view(sv, [[3 * 96, nr], [3, 31]], srow0 * 96 + 3))

        if m == 0:
            # emission order = tap consumption order (dyp 2 feeds taps t=0-2,
            # dyp 0 t=3-5, dyp 1 t=6-8); vector is the fastest copier and gets
            # the whole critical j=0 half, scalar frees up after its 4 weight
            # descriptors and matches the j=1 weight-DMA pacing, gpsimd (4x
            # slower) only gets the late-needed dxx=4 pieces
            for stg, j, dyp in ((st1, 0, 2), (st2, 0, 2), (st1, 0, 0),
                                (st2, 0, 0), (st1, 0, 1), (st2, 0, 1)):
                split_copy((nc.vector, nc.vector, nc.gpsimd), stg,
                           bx1 if stg is st1 else bx2, j, dyp)
            for stg, j, dyp in ((st1, 1, 2), (st2, 1, 2), (st1, 1, 0),
                                (st2, 1, 0), (st1, 1, 1), (st2, 1, 1)):
                split_copy((nc.scalar, nc.scalar, nc.gpsimd), stg,
                           bx1 if stg is st1 else bx2, j, dyp)
        else:
            # gpsimd takes ~2.4us per strided copy (vs ~0.15us on DVE) and
            # kept falling just behind the p-loop's dxx=4 reads, stalling the
            # PE one beat every ~8us; vector absorbs c1+c3 easily
            for j in range(2):
                for stg, band in ((st1, bx1), (st2, bx2)):
                    for dyp in range(3):
                        split_copy((nc.scalar, nc.vector, nc.vector),
                                   stg, band, j, dyp)
        for p in range(K2):
            dy, dx = p // 3, p % 3
            last_step = (m == 7 and p == K2 - 1)
            ckq1_ps = convps.tile([128, 2 * COUT], F32, tag="ckq1_ps")
            q2_ps = convps.tile([128, COUT], F32, tag="q2_ps")
            for j in range(2):
                for t in range(K2):
                    if j == 1 and t == K2 - 1 and not last_step:
                        # previous step's A^T matmuls slot in BEFORE the final
                        # conv tap: their LDWEIGHTS hide under these conv MMs
                        # and the p->p+1 conv boundary stays LDW-pipelined
                        # (a12 at the boundary cost ~110ns x ~56 boundaries)
                        flush_pend()
                    ty, tx = t // 3, t % 3
                    v = dy + ty - 1
                    s = _SROW[v]
                    dxx = dx + tx            # (dx+tx-1) + 1
                    lhs1 = bx1[:, j, dxx, s:s + 4, :]
                    lhs2 = bx2[:, j, dxx, s:s + 4, :]
                    st = (j == 0 and t == 0)
                    sp = (j == 1 and t == K2 - 1)
                    nc.tensor.matmul(ckq1_ps[:], lhs1,
                                     wkq1_t[:, j, t * 512:(t + 1) * 512], start=st, stop=sp)
                    nc.tensor.matmul(q2_ps[:], lhs2,
                                     wq2_t[:, j, t * COUT:(t + 1) * COUT], start=st, stop=sp)
            ckt = cpool.tile([128, COUT], BF16, tag="ckt")
            q12t = cpool.tile([128, 2 * COUT], BF16, tag="q12t")
            nc.vector.tensor_copy(ckt[:], ckq1_ps[:, 0:COUT])
            nc.vector.tensor_copy(q12t[:, 0:COUT], ckq1_ps[:, COUT:2 * COUT])
            if last_step:
                # third cast on the (idle) ACT engine: phase C's first matmul
                # waits all three convps readers, so parallelizing the casts
                # clears the pool barrier ~0.4us earlier
                nc.scalar.copy(q12t[:, COUT:2 * COUT], q2_ps[:])
            else:
                nc.vector.tensor_copy(q12t[:, COUT:2 * COUT], q2_ps[:])
            if last_step:
                # final step: p8's own casts were emitted on the DVE queue
                # FIRST (above), so the a12(p8) matmuls (and with them phase
                # C) aren't stuck behind p7's adds/mult on the DVE
                flush_pend()
            pend[0] = (m, p, ckt, q12t)
    flush_pend()


def _phase_c(nc, tc, ctx, x1, attn_p, u_out, cb, CBR):
    """U = conv(x1, attn) in natural [cout, pixel] layout.

    Row groups of 20 output rows (4 PSUM banks x 5 rows, N=480) + a final
    16-row group. j (cin chunk) is the outer accumulation loop so the first
    matmuls only depend on the j=0 half of attn.
    """
    spool = ctx.enter_context(tc.tile_pool(name="stage", bufs=2))
    # TWO single-buffer pools instead of one 8-bank pool: a pool's entry
    # barrier covers its whole extent, and an 8-bank pool would make the
    # very first phase-C matmul wait for the LAST phase-B PSUM reader (the
    # final a12 adds on aps's banks). The left pool only covers convps's
    # banks, whose casts finish ~2.5us earlier.
    cps_a = ctx.enter_context(tc.tile_pool(name="cps_a", bufs=1, space="PSUM",
                                           side="left"))
    cps_b = ctx.enter_context(tc.tile_pool(name="cps_b", bufs=1, space="PSUM",
                                           side="right"))

    GROUPS = [(0, 20), (20, 20), (40, 20), (60, 20), (80, 16)]
    for g, (gy, nrow) in enumerate(GROUPS):
        band = cb[g % 2]
        rb = nrow // 4                       # rows per PSUM bank (5 or 4)
        y0 = gy - 1
        r0 = max(0, -y0)
        r1 = min(CBR, 96 - y0, nrow + 2)
        if g == 4:
            nc.gpsimd.memset(band[:, :, nrow + 1, :], 0.0)
        if g >= 2:
            # groups 0/1 were prefetched during phase B
            for j in range(2):
                nc.sync.dma_start(band[:, j, r0:r1, 1:97],
                                  x1[j * 128:(j + 1) * 128, y0 + r0:y0 + r1, :])
        for i in range(2):
            ps = (cps_a if (g * 2 + i) % 2 == 0 else cps_b).tile(
                [128, 4, 512], F32, tag="cps")
            idx = 0
            for j in range(2):
                for t in range(K2):
                    ty, tx = t // 3, t % 3
                    lhsT = attn_p[t][:, j, i * 128:(i + 1) * 128]
                    st = (idx == 0)
                    sp = (idx == 2 * K2 - 1)
                    for b4 in range(4):
                        rr = rb * b4 + ty
                        rhs = band[:, j, rr:rr + rb, tx:tx + 96]
                        nc.tensor.matmul(ps[:, b4, 0:rb * 96], lhsT, rhs, start=st, stop=sp)
                    idx += 1
            stage = spool.tile([128, 1920], BF16, tag="stage")
            if g == len(GROUPS) - 1 and i == 1:
                # the very last output: copy+DMA per half PSUM bank so the DMA
                # of chunk c overlaps the copy of chunk c+1 (shrinks the tail)
                hb = rb * 96 // 2
                for c8 in range(8):
                    b4, h = c8 // 2, c8 % 2
                    sl = slice(b4 * rb * 96 + h * hb, b4 * rb * 96 + (h + 1) * hb)
                    # alternate DVE/ACT so the drain casts run in parallel
                    if c8 % 2 == 0:
                        nc.vector.tensor_copy(stage[:, sl], ps[:, b4, h * hb:(h + 1) * hb])
                    else:
                        nc.scalar.copy(stage[:, sl], ps[:, b4, h * hb:(h + 1) * hb])
                    nc.sync.dma_start(
                        u_out[i * 128:(i + 1) * 128,
                              (gy + b4 * rb) * 96 + h * hb:
                              (gy + b4 * rb) * 96 + (h + 1) * hb],
                        stage[:, sl])
            else:
                nc.vector.tensor_copy(stage[:, 0:nrow * 96], ps[:, :, 0:rb * 96])
                nc.sync.dma_start(u_out[i * 128:(i + 1) * 128, gy * 96:(gy + nrow) * 96],
                                  stage[:, 0:nrow * 96])


def _emit(nc, tc, x1, x2, wkq1, wq2, pos1t, pos2t, u_out, stats_out):
    CBR = 22
    with ExitStack() as octx:
        apool = octx.enter_context(tc.tile_pool(name="accum", bufs=1))
        atpool = octx.enter_context(tc.tile_pool(name="attn", bufs=1))
        spool2 = octx.enter_context(tc.tile_pool(name="stats", bufs=1))
        cbpool = octx.enter_context(tc.tile_pool(name="cbands", bufs=1))

        a1t = apool.tile([128, 2, K2 * COUT], F32, tag="a1t")
        a2t = apool.tile([128, 2, K2 * COUT], F32, tag="a2t")
        attn_p = [atpool.tile([128, 2, COUT], BF16, tag=f"attn_p{p}",
                              name=f"attn_p{p}") for p in range(K2)]
        cb = [cbpool.tile([128, 2, CBR, 98], BF16, tag=f"cb{i}", name=f"cb{i}")
              for i in range(2)]

        with ExitStack() as ctx:
            _phase_b(nc, tc, ctx, x1, x2, wkq1, wq2, a1t, a2t, pos1t, pos2t,
                     attn_p, cb, CBR)

        # stats off the critical path: they only feed the host-side affine,
        # not phase C, so they run on DVE/ACT while the PE grinds phase C
        stat = spool2.tile([128, 40], F32, tag="stat")
        sq = spool2.tile([128, COUT], BF16, tag="sq")
        for p in range(K2):
            for j in range(2):
                col = j * K2 + p
                nc.vector.reduce_sum(stat[:, col:col + 1], attn_p[p][:, j, :],
                                     axis=mybir.AxisListType.X)
                nc.scalar.activation(sq[:], attn_p[p][:, j, :],
                                     mybir.ActivationFunctionType.Square,
                                     accum_out=stat[:, 20 + col:21 + col])
        nc.sync.dma_start(stats_out[:], stat[:])

        with ExitStack() as ctx3:
            _phase_c(nc, tc, ctx3, x1, attn_p, u_out, cb, CBR)


def _prep_w(w):
    # [Cout, Cin, 3, 3] -> [cin, (ty*3+tx)*256 + cout]
    return np.ascontiguousarray(
        w.transpose(1, 2, 3, 0).reshape(CIN, K2 * COUT))


def _prep_pos(pos):
    # [Cout, Cin*9] -> [cin, p*256 + cout]
    return np.ascontiguousarray(
        pos.reshape(COUT, CIN, K2).transpose(1, 2, 0).reshape(CIN, K2 * COUT),
        dtype=np.float32)


def _run_cached(nc, in_maps):
    """Like run_bass_kernel_spmd under axon, but the shard_map jit is built
    once and cached so repeated kernel() calls skip re-trace/re-compile."""
    import jax
    from jax.sharding import Mesh, PartitionSpec
    from jax.experimental.shard_map import shard_map
    from concourse import bass2jax

    if "runner" not in _CACHE:
        bass2jax.install_neuronx_cc_hook()
        assert nc.dbg_addr is None
        pname = nc.partition_id_tensor.name if nc.partition_id_tensor else None
        in_names, out_names, out_avals, zero_shapes = [], [], [], []
        for alloc in nc.m.functions[0].allocations:
            if not isinstance(alloc, mybir.MemoryLocationSet):
                continue
            name = alloc.memorylocations[0].name
            if alloc.kind == "ExternalInput":
                if name != pname:
                    in_names.append(name)
            elif alloc.kind == "ExternalOutput":
                shape = tuple(alloc.tensor_shape)
                dtype = mybir.dt.np(alloc.dtype)
                out_names.append(name)
                out_avals.append(jax.core.ShapedArray(shape, dtype))
                zero_shapes.append((shape, dtype))
        n_params = len(in_names)
        all_names = in_names + out_names
        if pname is not None:
            all_names = all_names + [pname]
        donate = tuple(range(n_params, n_params + len(out_names)))

        def _body(*args):
            operands = list(args)
            if pname is not None:
                operands.append(bass2jax.partition_id_tensor())
            outs = bass2jax._bass_exec_p.bind(
                *operands,
                out_avals=tuple(out_avals),
                in_names=tuple(all_names),
                out_names=tuple(out_names),
                lowering_input_output_aliases=(),
                sim_require_finite=True,
                sim_require_nnan=True,
                nc=nc,
            )
            return tuple(outs)

        devices = jax.devices()[:N_CORES]
        mesh = Mesh(np.asarray(devices), ("core",))
        n_all = n_params + len(out_names)
        sharded = jax.jit(
            shard_map(_body, mesh=mesh,
                      in_specs=(PartitionSpec("core"),) * n_all,
                      out_specs=(PartitionSpec("core"),) * len(out_names),
                      check_rep=False),
            donate_argnums=donate, keep_unused=True)
        _CACHE["runner"] = (sharded, in_names, out_names, out_avals, zero_shapes)

    sharded, in_names, out_names, out_avals, zero_shapes = _CACHE["runner"]
    concat_in = [
        np.concatenate([np.asarray(in_maps[c][name]) for c in range(N_CORES)], axis=0)
        for name in in_names
    ]
    concat_zeros = [np.zeros((N_CORES * s[0], *s[1:]), dt) for s, dt in zero_shapes]
    out_arrs = sharded(*concat_in, *concat_zeros)
    return [
        {name: np.asarray(out_arrs[i]).reshape(N_CORES, *out_avals[i].shape)[c]
         for i, name in enumerate(out_names)}
        for c in range(N_CORES)
    ]


def build_in_maps(x1, x2, key_w, q1_w, q2_w, pos1, pos2):
    wk_r, wq1_r, wq2_r = _prep_w(key_w), _prep_w(q1_w), _prep_w(q2_w)
    wkq1_r = np.ascontiguousarray(
        np.concatenate([wk_r.reshape(CIN, K2, COUT), wq1_r.reshape(CIN, K2, COUT)],
                       axis=2).reshape(CIN, K2 * 2 * COUT)).astype(BF16_NP)
    wq2_b = wq2_r.astype(BF16_NP)
    p1_r, p2_r = _prep_pos(pos1), _prep_pos(pos2)
    x1b = x1.astype(BF16_NP)
    x2b = x2.astype(BF16_NP)
    return [{
        "x1": np.ascontiguousarray(x1b[b]),
        "x2": np.ascontiguousarray(x2b[b]),
        "wkq1": wkq1_r, "wq2": wq2_b,
        "pos1t": p1_r, "pos2t": p2_r,
    } for b in range(N_CORES)]


def kernel(**inputs):
    x1 = np.asarray(inputs["x1"], dtype=np.float32)
    x2 = np.asarray(inputs["x2"], dtype=np.float32)
    key_w = np.asarray(inputs["key_w"], dtype=np.float32)
    q1_w = np.asarray(inputs["q1_w"], dtype=np.float32)
    q2_w = np.asarray(inputs["q2_w"], dtype=np.float32)
    pos1 = np.asarray(inputs["pos1"], dtype=np.float32)
    pos2 = np.asarray(inputs["pos2"], dtype=np.float32)
    momentum = float(np.asarray(inputs["kernel_momentum"]))

    assert x1.shape == (B, CIN, H, W), x1.shape

    if "nc" not in _CACHE:
        _CACHE["nc"] = build_nc()
    nc = _CACHE["nc"]

    in_maps = build_in_maps(x1, x2, key_w, q1_w, q2_w, pos1, pos2)
    _CACHE["last_in_maps"] = in_maps
    results = _run_cached(nc, in_maps)

    U = np.stack([np.asarray(results[b]["u"]).astype(np.float32)
                  for b in range(N_CORES)])                          # [B, Cout, HW]
    stats = np.stack([results[b]["stats"] for b in range(N_CORES)])  # [B, 128, 4]

    s1 = float(stats[:, :, 0:2 * K2].astype(np.float64).sum())
    s2 = float(stats[:, :, 20:20 + 2 * K2].astype(np.float64).sum())
    n = float(B * COUT * CIN * K2)
    mu = s1 / n
    var = (s2 - s1 * s1 / n) / (n - 1.0)
    sd = np.sqrt(max(var, 0.0))
    scale = NORM_SCALE / (sd + 1e-4)
    a = momentum + scale
    bb = -mu * scale

    # boxsum(x1): conv(x1, all-ones weights) — identical for every out channel
    # (computed from the bf16-rounded x1 the device actually convolved)
    t = x1.astype(BF16_NP).astype(np.float32).sum(axis=1)            # [B, 96, 96]
    tp = np.pad(t, ((0, 0), (1, 1), (1, 1)))
    s = np.zeros_like(t)
    for ty in range(3):
        for tx in range(3):
            s += tp[:, ty:ty + 96, tx:tx + 96]

    out = a * U.reshape(B, COUT, H, W) + bb * s[:, None, :, :]
    return out.astype(np.float32)
